# revision 1
# baseline (speedup 1.0000x reference)
"""Correlation cost-volume (SpatialCorrelationSampler k=1, patch=9) + leaky ReLU.

Full inputs: feat1, feat2 [16, 256, 96, 160] f32.  Output [16, 81, 96, 160] f32.
corr[b, 9*i+j, y, x] = leaky_relu(sum_c f1[b,c,y,x] * f2[b,c,y+i-4,x+j-4], 0.1)

Strategy (8 NeuronCores, data-parallel over batch, 2 images/core):
  - per (image, 80-col half, row y): Gram-band matmuls on TensorE in bf16:
      lhsT = f1[c_chunk, y, x0:x0+80]            [K=128, M=80]
      rhs  = f2pad[c_chunk, y-4..y+4, x0-4..x0+83] -> [K=128, 9*88] contiguous
    2 C-chunks accumulate in PSUM (dys 0..4 -> psum[:,0:440], dys 5..8 -> [512:864],
    each region inside one PSUM bank).
  - ScalarE evicts PSUM with Prelu(alpha=0.1) into S[80, 792] in *interleaved*
    layout col = 9*x' + dy.  Then the 81 band values of partition m are the
    contiguous run S[m, 9m : 9m+81] (ordered k = 9*dx + dy).
  - One skewed DMA per row (step = rowlen+9 across partitions, 81-contig runs)
    writes the band straight to DRAM; host reorders the 81 channels + x tiles.
"""

import numpy as np

import bass_rust
import concourse.bacc as bacc
import concourse.bass as bass
import concourse.mybir as mybir
import concourse.tile as tile
from concourse.bass_utils import run_bass_kernel_spmd

B, C, H, W = 16, 256, 96, 160
NCORES = 8
NB = B // NCORES          # images per core
WH = 80                   # column-half width (matmul M)
WPAD = WH + 8             # rhs window width
HPAD = H + 8              # zero-padded rows
NPATCH = 81


_LRELU_OP = None


def _get_lrelu_op():
    """Register a one-pass leaky-relu custom DVE op: out = max(x, x*imm2)."""
    global _LRELU_OP
    if _LRELU_OP is not None:
        return _LRELU_OP
    from concourse import dve_ops as dops
    from concourse.dve_spec import Spec, Src0, C2, maxx, lower
    from concourse.dve_uop import DveOpSpec
    name = "LRELU_ANT"
    if name in dops.CUSTOM_DVE_SPECS:
        _LRELU_OP = next(o for o in dops.OPS if o.name == name)
        return _LRELU_OP
    spec = Spec(
        body=maxx(Src0, Src0 * C2),
        reference=lambda in0, in1, c0, c1, c2: np.maximum(in0, in0 * c2))
    opcode = dops._CUSTOM_DVE_ROW_BASE + len(dops.OPS)
    shas = {}
    for ver in ("v3", "v4"):
        try:
            o = DveOpSpec(name=name, opcode=opcode, uops=lower(spec, ver=ver),
                          rd1_en=False)
            shas[ver] = o.sha(ver)
        except Exception:
            pass
    op = dops.DveOp(name, spec, subdim=False, uops_sha=shas)
    dops.OPS.append(op)
    dops.CUSTOM_DVE_SPECS[name] = spec
    dops._SUB_OPCODE_FOR_NAME[name] = opcode
    _LRELU_OP = op
    return op


def build_nc(leaky: bool = True, units: list | None = None,
             s_bufs: int = 4, ps_rows: int = 1,
             diag_eng: str = 'sync', srows: int = 6) -> bass.Bass:
    lrelu_op = _get_lrelu_op()
    nc = bacc.Bacc()
    f1_ext = nc.declare_dram_parameter(
        "feat1", [NB, C, H, W], mybir.dt.float32, isOutput=False)
    f2_ext = nc.declare_dram_parameter(
        "feat2", [NB, C, H, W], mybir.dt.float32, isOutput=False)
    out_ext = nc.declare_dram_parameter(
        "out", [NB, 2, H // 4, WH, 4 * NPATCH], mybir.dt.float32, isOutput=True)

    act_fn = (mybir.ActivationFunctionType.Prelu if leaky
              else mybir.ActivationFunctionType.Relu)
    neg = 0.1 if leaky else 0.0
    if units is None:
        units = [(b, v) for b in range(NB) for v in range(2)]

    HU = H // 2           # rows per vertical-half unit (48)
    HU2 = HU + 8          # padded rows held per unit (56)
    W2 = W + 8            # f2 padded width (168)

    with tile.TileContext(nc) as tc:
        with (
            tc.tile_pool(name="feat", bufs=2) as featp,
            tc.tile_pool(name="spool", bufs=s_bufs) as spool,
            tc.tile_pool(name="psum", bufs=8 // (2 * ps_rows), space="PSUM") as psump,
        ):
            for (b, v) in units:
                y0 = HU * v
                f1u = [featp.tile([128, HU * W], mybir.dt.bfloat16,
                                  tag=f"f1u{c}", name=f"f1u{c}_{b}_{v}")
                       for c in range(2)]
                f2u = [featp.tile([128, HU2 * W2], mybir.dt.bfloat16,
                                  tag=f"f2u{c}", name=f"f2u{c}_{b}_{v}")
                       for c in range(2)]
                # zero the pad borders every unit (no assumption about
                # which pool slot this tile landed in)
                for c in range(2):
                    f2v = f2u[c][:, :].rearrange("p (r w) -> p r w", w=W2)
                    if v == 0:
                        nc.gpsimd.memset(f2v[:, 0:4, :], 0.0)
                    else:
                        nc.gpsimd.memset(f2v[:, HU2 - 4:HU2, :], 0.0)
                    nc.gpsimd.memset(f2v[:, :, 0:4], 0.0)
                    nc.gpsimd.memset(f2v[:, :, W2 - 4:W2], 0.0)

                # SWDGE casting loads (f32 DRAM -> bf16 SBUF), interleaved
                # strips so the first rows' operands arrive first
                a_lo = max(0, y0 - 4)
                a_hi = min(H, y0 + HU + 4)
                f2starts = list(range(a_lo, a_hi, srows))
                f1starts = list(range(0, HU, srows))
                for i in range(max(len(f1starts), len(f2starts))):
                    for c in range(2):
                        f2v = f2u[c][:, :].rearrange("p (r w) -> p r w", w=W2)
                        if i < len(f2starts):
                            t0 = f2starts[i]
                            t1 = min(t0 + srows, a_hi)
                            r0 = t0 + 4 - y0
                            nc.gpsimd.dma_start(
                                f2v[:, r0:r0 + (t1 - t0), 4:4 + W],
                                f2_ext[b, 128 * c:128 * (c + 1), t0:t1, :])
                        if i < len(f1starts):
                            s0 = f1starts[i]
                            nc.gpsimd.dma_start(
                                f1u[c][:, s0 * W:(s0 + srows) * W],
                                f1_ext[b, 128 * c:128 * (c + 1),
                                       y0 + s0:y0 + s0 + srows, :])

                SW = 9 * WPAD                # 792, one row's S width
                for h in range(2):
                    x0 = WH * h
                    for y4 in range(0, HU, 4):
                        S = spool.tile([WH, 4 * SW], mybir.dt.float32, tag="S",
                                       name=f"S_{b}_{v}_{h}_{y4}")
                        Sb = S[:, :]
                        RL = Sb.ap[0][0]         # 4*SW
                        for y2 in range(y4, y4 + 4, ps_rows):
                            ps = psump.tile([WH, 1024 * ps_rows],
                                            mybir.dt.float32, tag="ps",
                                            name=f"ps_{b}_{v}_{h}_{y2}")
                            for c in range(2):
                                f2b_ = f2u[c][:, :]
                                F2RL = f2b_.ap[0][0]
                                for r in range(ps_rows):
                                    yl = y2 + r
                                    lhs = f1u[c][:, yl * W + x0:yl * W + x0 + WH]
                                    rhs1 = bass_rust.AP(
                                        f2b_.tensor, f2b_.offset + yl * W2 + x0,
                                        [[F2RL, 128], [W2, 5], [1, WPAD]])
                                    rhs2 = bass_rust.AP(
                                        f2b_.tensor,
                                        f2b_.offset + (yl + 5) * W2 + x0,
                                        [[F2RL, 128], [W2, 4], [1, WPAD]])
                                    nc.tensor.matmul(
                                        ps[0:WH, r * 1024:r * 1024 + 440],
                                        lhs, rhs1,
                                        start=(c == 0), stop=(c == 1))
                                    nc.tensor.matmul(
                                        ps[0:WH, r * 1024 + 512:r * 1024 + 864],
                                        lhs, rhs2,
                                        start=(c == 0), stop=(c == 1))
                            # evict + leaky relu into interleaved S:
                            # S col = 36*x' + 9*row_in_batch + dy.
                            # ACT takes dys 0..4 (both rows, one instr),
                            # DVE takes dys 5..8 (custom lrelu, one per row).
                            so = Sb.offset + 9 * (y2 - y4)
                            psb = ps[:, :]
                            PRL = 1024 * ps_rows
                            dst1 = bass_rust.AP(
                                Sb.tensor, so,
                                [[RL, WH], [9, ps_rows], [1, 5], [36, WPAD]])
                            src1 = bass_rust.AP(
                                psb.tensor, psb.offset,
                                [[PRL, WH], [1024, ps_rows], [WPAD, 5], [1, WPAD]])
                            nc.scalar.activation(dst1, src1, act_fn, alpha=neg)
                            for r in range(ps_rows):
                                dst2 = bass_rust.AP(
                                    Sb.tensor, so + 9 * r + 5,
                                    [[RL, WH], [1, 4], [36, WPAD]])
                                src2 = bass_rust.AP(
                                    psb.tensor, psb.offset + r * 1024 + 512,
                                    [[PRL, WH], [WPAD, 4], [1, WPAD]])
                                nc.vector._custom_dve(lrelu_op, out=dst2,
                                                      in0=src2, imm2=neg)

                        # band DMA: partition m reads contiguous
                        # S[m, 36m : 36m + 324]
                        diag = bass_rust.AP(Sb.tensor, Sb.offset,
                                            [[RL + 36, WH], [1, 4 * NPATCH]])
                        getattr(nc, diag_eng).dma_start(
                            out_ext[b, h, (y0 + y4) // 4], diag)
    nc.finalize()
    return nc


_CACHE: dict = {}


def _get_nc() -> bass.Bass:
    if "nc" not in _CACHE:
        _CACHE["nc"] = build_nc(leaky=True)
    return _CACHE["nc"]


def _assemble(core_outs: list) -> np.ndarray:
    # device layout: [b, h, yblk(24), m(80), dx(9), r(4), dy(9)]
    # reference:     [b, 9*dy + dx, 4*yblk + r, 80*h + m]
    full = np.empty((B, NPATCH, H, W), dtype=np.float32)
    for i, a in enumerate(core_outs):
        a = np.asarray(a).reshape(NB, 2, H // 4, WH, 9, 4, 9)
        full[NB * i:NB * (i + 1)] = (
            a.transpose(0, 6, 4, 2, 5, 1, 3).reshape(NB, NPATCH, H, W))
    return full


def kernel(feat1: np.ndarray, feat2: np.ndarray, **_ignored) -> np.ndarray:
    feat1 = np.ascontiguousarray(np.asarray(feat1), dtype=np.float32)
    feat2 = np.ascontiguousarray(np.asarray(feat2), dtype=np.float32)
    nc = _get_nc()
    in_maps = [
        {"feat1": feat1[NB * i:NB * (i + 1)], "feat2": feat2[NB * i:NB * (i + 1)]}
        for i in range(NCORES)
    ]
    res = run_bass_kernel_spmd(nc, in_maps, list(range(NCORES)))
    return _assemble([res.results[i]["out"] for i in range(NCORES)])



# revision 9
# speedup vs baseline: 1.3621x; 1.3621x over previous
"""Correlation cost-volume (SpatialCorrelationSampler k=1, patch=9) + leaky ReLU.

Full inputs: feat1, feat2 [16, 256, 96, 160] f32.  Output [16, 81, 96, 160] f32.
corr[b, 9*i+j, y, x] = leaky_relu(sum_c f1[b,c,y,x] * f2[b,c,y+i-4,x+j-4], 0.1)

Strategy (8 NeuronCores, data-parallel over batch, 2 images/core), v3:
  - per (image, 80-col half, row y): Gram-band matmuls on TensorE in bf16:
      lhsT = f1[c_chunk, y, x0:x0+80]            [K=128, M=80]
      rhs  = f2[c_chunk, y-4..y+4, x0-4..x0+83] -> [K=128, 9*88] windows
    2 C-chunks accumulate in PSUM; 2 rows per PSUM tile (4 banks, 2 bufs).
  - f2 is staged UNPADDED in x ([128, 4 | 56*160 | 4] bf16 with 4-elem
    slack): windows at the x edges read row-wrapped garbage that only lands
    in out-of-bounds displacement outputs, which the host zeroes.  This
    makes every input load a big contiguous SWDGE casting DMA
    (f32 DRAM -> bf16 SBUF), ~2 MB a piece, instead of 320-byte
    read-modify-write descriptor strips.  y pad rows are memset zero.
  - Eviction PSUM -> SBUF with leaky-relu, bf16: one ACT instr per 2 rows
    (dys 0..4), one custom-DVE lrelu per row (dys 5..8), interleaved S
    layout col = 36*dx + 9*r + dy so partition m's 4-row band is the
    contiguous 324-elem run S[m, 36m : 36m+324].
  - One skewed band DMA per (row-quad, half) (partition pitch RL+36),
    bf16, alternating sync/scalar HWDGE rings; host reorders channels,
    zeroes x-edge OOB entries, casts to f32.
"""

import numpy as np

import bass_rust
import concourse.bacc as bacc
import concourse.bass as bass
import concourse.mybir as mybir
import concourse.tile as tile
from concourse.bass_utils import run_bass_kernel_spmd

B, C, H, W = 16, 256, 96, 160
NCORES = 8
NB = B // NCORES          # images per core
WH = 80                   # column-half width (matmul M)
WPAD = WH + 8             # rhs window width
NPATCH = 81
HU = H // 2               # rows per vertical-half unit (48)
HU2 = HU + 8              # f2 rows held per unit (56)
F2W = 4 + HU2 * W + 4     # f2 tile width: 4-elem slack each side (8968)
SW = 36 * WPAD            # S cols per y4 tile (3168): col = 36*dx + 9*r + dy


_LRELU_OP = None


def _get_lrelu_op():
    """Register a one-pass leaky-relu custom DVE op: out = max(x, x*imm2)."""
    global _LRELU_OP
    if _LRELU_OP is not None:
        return _LRELU_OP
    from concourse import dve_ops as dops
    from concourse.dve_spec import Spec, Src0, C2, maxx, lower
    from concourse.dve_uop import DveOpSpec
    name = "LRELU_ANT"
    if name in dops.CUSTOM_DVE_SPECS:
        _LRELU_OP = next(o for o in dops.OPS if o.name == name)
        return _LRELU_OP
    spec = Spec(
        body=maxx(Src0, Src0 * C2),
        reference=lambda in0, in1, c0, c1, c2: np.maximum(in0, in0 * c2))
    opcode = dops._CUSTOM_DVE_ROW_BASE + len(dops.OPS)
    shas = {}
    for ver in ("v3", "v4"):
        try:
            o = DveOpSpec(name=name, opcode=opcode, uops=lower(spec, ver=ver),
                          rd1_en=False)
            shas[ver] = o.sha(ver)
        except Exception:
            pass
    op = dops.DveOp(name, spec, subdim=False, uops_sha=shas)
    dops.OPS.append(op)
    dops.CUSTOM_DVE_SPECS[name] = spec
    dops._SUB_OPCODE_FOR_NAME[name] = opcode
    _LRELU_OP = op
    return op


def build_nc(leaky: bool = True, units: list | None = None,
             s_bufs: int = 4, ps_rows: int = 1) -> bass.Bass:
    lrelu_op = _get_lrelu_op()
    nc = bacc.Bacc()
    f1_ext = nc.declare_dram_parameter(
        "feat1", [NB, C, H, W], mybir.dt.float32, isOutput=False)
    f2_ext = nc.declare_dram_parameter(
        "feat2", [NB, C, H, W], mybir.dt.float32, isOutput=False)
    out_ext = nc.declare_dram_parameter(
        "out", [NB, 2, H // 4, WH, 4 * NPATCH], mybir.dt.bfloat16,
        isOutput=True)

    act_fn = (mybir.ActivationFunctionType.Prelu if leaky
              else mybir.ActivationFunctionType.Relu)
    neg = 0.1 if leaky else 0.0
    if units is None:
        units = [(b, v) for b in range(NB) for v in range(2)]

    with tile.TileContext(nc) as tc:
        with (
            tc.tile_pool(name="feat", bufs=2) as featp,
            tc.tile_pool(name="spool", bufs=s_bufs) as spool,
            tc.tile_pool(name="psum", bufs=8 // (2 * ps_rows),
                         space="PSUM") as psump,
        ):
            for (b, v) in units:
                y0 = HU * v
                f1u = [featp.tile([128, HU * W], mybir.dt.bfloat16,
                                  tag=f"f1u{c}", name=f"f1u{c}_{b}_{v}")
                       for c in range(2)]
                f2u = [featp.tile([128, F2W], mybir.dt.bfloat16,
                                  tag=f"f2u{c}", name=f"f2u{c}_{b}_{v}")
                       for c in range(2)]
                a_lo = max(0, y0 - 4)
                a_hi = min(H, y0 + HU + 4)        # 52 DRAM rows per unit
                # y-pad rows (zero); x edges handled by host zeroing
                for c in range(2):
                    if v == 0:
                        nc.gpsimd.memset(f2u[c][:, 4:4 + 4 * W], 0.0)
                    else:
                        nc.gpsimd.memset(
                            f2u[c][:, 4 + 52 * W:4 + 56 * W], 0.0)
                # big contiguous SWDGE casting loads, 2 strips per tensor
                # per chunk; first strips of both chunks first
                f2starts = [a_lo, a_lo + 26]
                f1starts = [0, 24]
                for i in range(2):
                    for c in range(2):
                        t0 = f2starts[i]
                        t1 = min(t0 + 26, a_hi)
                        d0 = t0 + 4 - y0 if v == 0 else t0 - 44
                        nc.gpsimd.dma_start(
                            f2u[c][:, 4 + d0 * W:4 + (d0 + (t1 - t0)) * W],
                            f2_ext[b, 128 * c:128 * (c + 1), t0:t1, :])
                        s0 = f1starts[i]
                        nc.gpsimd.dma_start(
                            f1u[c][:, s0 * W:(s0 + 24) * W],
                            f1_ext[b, 128 * c:128 * (c + 1),
                                   y0 + s0:y0 + s0 + 24, :])

                nout = 0
                for h in range(2):
                    x0 = WH * h
                    for y4 in range(0, HU, 4):
                        S = spool.tile([WH, SW], mybir.dt.bfloat16, tag="S",
                                       name=f"S_{b}_{v}_{h}_{y4}")
                        Sb = S[:, :]
                        RL = Sb.ap[0][0]
                        for y2 in range(y4, y4 + 4, ps_rows):
                            ps = psump.tile([WH, 1024 * ps_rows],
                                            mybir.dt.float32, tag="ps",
                                            name=f"ps_{b}_{v}_{h}_{y2}")
                            psb = ps[:, :]
                            PRL = psb.ap[0][0]
                            for c in range(2):
                                f2b = f2u[c][:, :]
                                F2RL = f2b.ap[0][0]
                                for r in range(ps_rows):
                                    yl = y2 + r
                                    lhs = f1u[c][:, yl * W + x0:
                                                 yl * W + x0 + WH]
                                    rhs1 = bass_rust.AP(
                                        f2b.tensor,
                                        f2b.offset + yl * W + x0,
                                        [[F2RL, 128], [W, 4], [1, WPAD]])
                                    rhs2 = bass_rust.AP(
                                        f2b.tensor,
                                        f2b.offset + (yl + 4) * W + x0,
                                        [[F2RL, 128], [W, 5], [1, WPAD]])
                                    nc.tensor.matmul(
                                        ps[0:WH, r * 1024:r * 1024 + 352],
                                        lhs, rhs1,
                                        start=(c == 0), stop=(c == 1))
                                    nc.tensor.matmul(
                                        ps[0:WH,
                                           r * 1024 + 512:r * 1024 + 952],
                                        lhs, rhs2,
                                        start=(c == 0), stop=(c == 1))
                            # evict + leaky relu into interleaved S:
                            # col = 36*dx + 9*r + dy.  ACT takes dys 0..3,
                            # DVE (custom lrelu) takes dys 4..8 — balanced
                            # for the 1.2 vs 0.96 GHz engine clocks and
                            # ACT's fixed PSUM-accumulator-read tax.
                            so = Sb.offset + 9 * (y2 - y4)
                            dst1 = bass_rust.AP(
                                Sb.tensor, so,
                                [[RL, WH], [9, ps_rows], [1, 4], [36, WPAD]])
                            src1 = bass_rust.AP(
                                psb.tensor, psb.offset,
                                [[PRL, WH], [1024, ps_rows], [WPAD, 4],
                                 [1, WPAD]])
                            nc.scalar.activation(dst1, src1, act_fn,
                                                 alpha=neg)
                            for r in range(ps_rows):
                                dst2 = bass_rust.AP(
                                    Sb.tensor, so + 9 * r + 4,
                                    [[RL, WH], [1, 5], [36, WPAD]])
                                src2 = bass_rust.AP(
                                    psb.tensor,
                                    psb.offset + r * 1024 + 512,
                                    [[PRL, WH], [WPAD, 5], [1, WPAD]])
                                nc.vector._custom_dve(lrelu_op, out=dst2,
                                                      in0=src2, imm2=neg)

                        # band DMA: partition m reads contiguous
                        # S[m, 36m : 36m + 324]
                        diag = bass_rust.AP(Sb.tensor, Sb.offset,
                                            [[RL + 36, WH], [1, 4 * NPATCH]])
                        eng = (nc.sync, nc.scalar)[nout % 2]
                        nout += 1
                        eng.dma_start(out_ext[b, h, (y0 + y4) // 4], diag)
    nc.finalize()
    return nc


_CACHE: dict = {}


def _get_nc() -> bass.Bass:
    if "nc" not in _CACHE:
        _CACHE["nc"] = build_nc(leaky=True)
    return _CACHE["nc"]


def _assemble(core_outs: list) -> np.ndarray:
    # device layout: [b, h, yblk(24), m(80), dx(9), r(4), dy(9)] bf16
    # reference:     [b, 9*dy + dx, 4*yblk + r, 80*h + m]
    full = np.empty((B, NPATCH, H, W), dtype=np.float32)
    for i, a in enumerate(core_outs):
        a = np.asarray(a).astype(np.float32)
        a = a.reshape(NB, 2, H // 4, WH, 9, 4, 9)
        full[NB * i:NB * (i + 1)] = (
            a.transpose(0, 6, 4, 2, 5, 1, 3).reshape(NB, NPATCH, H, W))
    # zero out-of-bounds x displacements (f2 col = x + dx - 4 outside [0, W))
    for x in range(4):
        ts = [9 * dy + t for dy in range(9) for t in range(4 - x)]
        full[:, ts, :, x] = 0.0
    for x in range(W - 4, W):
        ts = [9 * dy + t for dy in range(9) for t in range(164 - x, 9)]
        full[:, ts, :, x] = 0.0
    return full


def kernel(feat1: np.ndarray, feat2: np.ndarray, **_ignored) -> np.ndarray:
    feat1 = np.ascontiguousarray(np.asarray(feat1), dtype=np.float32)
    feat2 = np.ascontiguousarray(np.asarray(feat2), dtype=np.float32)
    nc = _get_nc()
    in_maps = [
        {"feat1": feat1[NB * i:NB * (i + 1)], "feat2": feat2[NB * i:NB * (i + 1)]}
        for i in range(NCORES)
    ]
    res = run_bass_kernel_spmd(nc, in_maps, list(range(NCORES)))
    return _assemble([res.results[i]["out"] for i in range(NCORES)])
